# revision 31
# baseline (speedup 1.0000x reference)
"""BioWaveKAN fused kernel for 8 Trainium2 NeuronCores.

y = wavelet(x) @ (pi^-1/4 * Ww).T + x @ (0.3 * Wb).T   (single K=4096 contraction)
out = BatchNorm1d(y)  (training-mode batch stats, all-reduced across cores)

Sharding: data-parallel over batch (8 x 512 rows). Device layout is transposed
(features on partitions); host pre-arranges x/weights into per-partition
contiguous DMA layouts and post-transposes the fp16 output.

v2 structure per core (vs v1: two-pass h0/h1 with a late stats AllReduce):
  - full-K (32 k-tile) PSUM accumulation per o-tile: one drain instead of two
  - o-tiles in 3 groups of [8, 6, 2] (PSUM banks); per-group stats AllReduce +
    finalize + normalize + store interleaved so only the last (2-tile) group's
    collective is exposed in the tail
  - warm-up matmuls keep the PE HAM un-throttled through the DMA fill
  - wavelet: DVE (t, k magic range-reduce, r, sin*exp product) + ACT (Square,
    Exp, then a single-table-switch Sin pass; Sqrt table prefetched);
    GpSimd is avoided for big tiles (~9 G elem/s measured on [128,512])
  - drains fully on DVE (copy+sum / square+sumsq via accum_out), keeping ACT
    off the tail and avoiding activation-table thrash
"""
import math

import numpy as np

from concourse import bacc
import concourse.tile as tile
import concourse.mybir as mybir
from concourse.bass_utils import run_bass_kernel_spmd

F32 = mybir.dt.float32
F16 = mybir.dt.float16
F8E3 = mybir.dt.float8e3
AF = mybir.ActivationFunctionType
OP = mybir.AluOpType
WSCALE = 64.0     # weights stored e3m4 scaled by 64 (max |w*64| ~ 7.8 < 15.5);
                  # un-scaled in the PSUM drain, exactness absorbed by BN

B = 4096          # batch
D = 2048          # in_dim == out_dim
NCORES = 8
BS = B // NCORES  # batch shard per core (512)
NIT = D // 128    # i-tiles (16)
NKT = 2 * NIT     # contraction tiles (32): 0..15 = x, 16..31 = wavelet
NOT = D // 128    # o-tiles (16)
BN_EPS = 1e-5
TWO_PI = 2.0 * math.pi
MAGIC = 1.5 * 2.0 ** 23

# o-tile groups: (o_start, group_size, stats col base); last group small so the
# tail (drain -> AllReduce -> finalize -> store) after the final matmul is short
GROUPS = [(0, 8, 0), (8, 6, 16), (14, 2, 28)]
NWCH = 8          # weight chunks per group (4 k-tiles each)

_CACHE = {}


def _build_nc():
    nc = bacc.Bacc()

    xh_d = nc.dram_tensor("xh", (128, NIT * BS), F16, kind="ExternalInput")
    w_d = [
        nc.dram_tensor(f"w{g}", (NWCH * 128, 4 * 128 * gs), F8E3,
                       kind="ExternalInput")
        for g, (_, gs, _) in enumerate(GROUPS)
    ]
    cst_d = nc.dram_tensor("cst", (128, 6 * NIT), F32, kind="ExternalInput")
    yT_d = nc.dram_tensor("yT", (128, NOT * BS), F16, kind="ExternalOutput")

    with tile.TileContext(nc) as tc:
        with (
            tc.tile_pool(name="big", bufs=1) as big,
            tc.tile_pool(name="small", bufs=1) as small,
            tc.tile_pool(name="wp0", bufs=6) as wp0,
            tc.tile_pool(name="wp1", bufs=8) as wp1,
            tc.tile_pool(name="wp2", bufs=8) as wp2,
            tc.tile_pool(name="scr", bufs=4) as scr,
            tc.tile_pool(name="qtp", bufs=3) as qtp,
            tc.tile_pool(name="sinp", bufs=2) as sinp,
            tc.tile_pool(name="dscp", bufs=2) as dscp,
            tc.tile_pool(name="ps", bufs=8, space="PSUM") as ps,
            tc.tile_pool(name="dram", bufs=1, space="DRAM") as dram,
        ):
            rhs = big.tile([128, NKT, BS], F16)   # 0..15 x, 16..31 wavelet
            y16 = big.tile([128, NOT, BS], F16)
            et = big.tile([128, NIT, BS], F16)    # exp(-u^2/2) per i-tile
            cstt = small.tile([128, 6 * NIT], F32)
            stats = small.tile([128, 32], F32)    # per group: sums then sumsqs
            red = small.tile([128, 32], F32)
            ab = small.tile([128, 32], F32)       # a cols 0..15, b cols 16..31
            wz = small.tile([128, 64], F16)

            zbt = small.tile([128, 1], F32)
            nc.vector.memset(zbt[:], 0.0)
            epst = small.tile([128, 1], F32)
            nc.vector.memset(epst[:], BN_EPS)

            # ---- PE warm-up: keep HAM at K=8/8 through the DMA fill ----
            nc.vector.memset(wz[:], 0.0)
            psw = ps.tile([128, BS], F32, tag="ps", name="psw")
            for _ in range(80):
                nc.tensor.matmul(psw[0:1, 0:64], wz[:, 0:1], wz[:, 0:64],
                                 start=True, stop=True)

            # ---- DMA issue order: consts, x, weights in consumption order ----
            nc.sync.dma_start(cstt[:], cst_d[:])
            s3t = cstt[:, 0 * NIT:1 * NIT]
            b3t = cstt[:, 1 * NIT:2 * NIT]
            sut = cstt[:, 2 * NIT:3 * NIT]
            but = cstt[:, 3 * NIT:4 * NIT]
            gmt = cstt[:, 4 * NIT:5 * NIT]
            btt = cstt[:, 5 * NIT:6 * NIT]

            xh_t = xh_d[:].rearrange("p (kt b) -> p kt b", b=BS)
            nc.sync.dma_start(rhs[:, 0:4, :], xh_t[:, 0:4, :])

            wtiles = {g: [] for g in range(len(GROUPS))}

            def issue_w(g, c, eng):
                gs = GROUPS[g][1]
                wt = locals_pools[g].tile([128, 4, 128 * gs], F8E3,
                                          tag=f"w{g}", name=f"w{g}_{c}")
                eng.dma_start(
                    wt[:], w_d[g][c * 128:(c + 1) * 128, :].rearrange(
                        "p (k o) -> p k o", k=4))
                wtiles[g].append(wt)

            locals_pools = {0: wp0, 1: wp1, 2: wp2}
            issue_w(0, 0, nc.sync)
            for c in range(1, 4):
                nc.sync.dma_start(rhs[:, c * 4:(c + 1) * 4, :],
                                  xh_t[:, c * 4:(c + 1) * 4, :])
            for c in range(1, NWCH):
                issue_w(0, c, nc.sync)
            for c in range(NWCH):
                issue_w(1, c, nc.sync)
            for c in range(NWCH):
                issue_w(2, c, nc.sync)

            # bridge warm-ups: depend on the x DMA so they run while the
            # first weight chunk is still in flight, keeping HAM at 8/8
            # (a >3.4us PE idle gap re-throttles the PE clock to 4/8)
            for j in range(40):
                nc.tensor.matmul(psw[0:1, 0:64], wz[:, 0:1],
                                 rhs[:, j % 4, 0:64], start=True, stop=True)

            # ---- wavelet: rhs[:,16+i,:] = sin(2*pi*frac(t)) * exp(-u^2/2) ----
            # t,k (magic-number range reduction), r on DVE;
            # Square/Exp then Sin on ACT (one table switch)
            for i in range(NIT):
                xf = rhs[:, i, :]
                tt_ = scr.tile([128, BS], F32, tag="scr", name=f"t_{i}")
                nc.vector.tensor_scalar(out=tt_[:], in0=xf,
                                        scalar1=s3t[:, i:i + 1],
                                        scalar2=b3t[:, i:i + 1],
                                        op0=OP.mult, op1=OP.add)
                kt_ = scr.tile([128, BS], F32, tag="scr", name=f"k_{i}")
                nc.vector.tensor_scalar(out=kt_[:], in0=tt_[:],
                                        scalar1=MAGIC, scalar2=MAGIC,
                                        op0=OP.add, op1=OP.subtract)
                nc.vector.tensor_tensor(rhs[:, NIT + i, :], tt_[:], kt_[:],
                                        op=OP.subtract)
                qt = qtp.tile([128, BS], F32, tag="qt", name=f"q_{i}")
                nc.scalar.activation(qt[:], xf, AF.Square,
                                     bias=but[:, i:i + 1], scale=sut[:, i:i + 1])
                nc.scalar.activation(et[:, i, :], qt[:], AF.Exp,
                                     bias=zbt[:], scale=-0.5)
            for i in range(NIT):
                sint = sinp.tile([128, BS], F16, tag="sin", name=f"s_{i}")
                nc.scalar.activation(sint[:], rhs[:, NIT + i, :], AF.Sin,
                                     bias=zbt[:], scale=TWO_PI)
                nc.vector.tensor_tensor(rhs[:, NIT + i, :], sint[:],
                                        et[:, i, :], op=OP.mult)
            # prefetch the sqrt table set off the critical path
            sqp = small.tile([128, 1], F32)
            nc.scalar.activation(sqp[:], zbt[:], AF.Sqrt, bias=epst[:])

            ibs, obs = {}, {}
            for g, (_, gs, _) in enumerate(GROUPS):
                ibs[g] = dram.tile([128, 2 * gs], F32, name=f"ib{g}")
                obs[g] = dram.tile([128, 2 * gs], F32, name=f"ob{g}")

            def mm_group(g):
                o0, gs, cb = GROUPS[g]
                psums = []
                for mo in range(gs):
                    pst = ps.tile([128, BS], F32, tag="ps",
                                  name=f"ps_{g}_{mo}")
                    psums.append(pst)
                for kt in range(NKT):
                    ch = wtiles[g][kt // 4]
                    ktl = kt % 4
                    for mo in range(gs):
                        nc.tensor.matmul(
                            psums[mo][:],
                            ch[:, ktl, mo * 128:(mo + 1) * 128],
                            rhs[:, kt, :],
                            start=(kt == 0), stop=(kt == NKT - 1))
                # fused drains on DVE: y16 = psum (fp16) + sum / sumsq accums
                for mo in range(gs):
                    m = o0 + mo
                    nc.vector.tensor_scalar(
                        out=y16[:, m, :], in0=psums[mo][:],
                        scalar1=1.0 / WSCALE, scalar2=0.0,
                        op0=OP.mult, op1=OP.add,
                        accum_out=stats[:, cb + mo:cb + mo + 1])
                    dsc = dscp.tile([128, BS], F16, tag="dsc", name=f"d_{m}")
                    nc.vector.scalar_tensor_tensor(
                        out=dsc[:], in0=y16[:, m, :], scalar=1.0,
                        in1=y16[:, m, :], op0=OP.mult, op1=OP.mult,
                        accum_out=stats[:, cb + gs + mo:cb + gs + mo + 1])
                nc.sync.dma_start(ibs[g][:], stats[:, cb:cb + 2 * gs])
                nc.gpsimd.collective_compute(
                    "AllReduce", OP.add,
                    replica_groups=[list(range(NCORES))],
                    ins=[ibs[g].opt()], outs=[obs[g].opt()])

            def fin_group(g):
                o0, gs, cb = GROUPS[g]
                rs = red[:, cb:cb + gs]          # sums
                rq = red[:, cb + gs:cb + 2 * gs]  # sumsqs
                mean = small.tile([128, gs], F32, name=f"mean{g}")
                nc.vector.tensor_scalar(out=mean[:], in0=rs, scalar1=1.0 / B,
                                        scalar2=None, op0=OP.mult)
                var = small.tile([128, gs], F32, name=f"var{g}")
                nc.vector.tensor_scalar(out=var[:], in0=rq, scalar1=1.0 / B,
                                        scalar2=None, op0=OP.mult)
                msq = small.tile([128, gs], F32, name=f"msq{g}")
                nc.vector.tensor_tensor(msq[:], mean[:], mean[:], op=OP.mult)
                nc.vector.tensor_tensor(var[:], var[:], msq[:], op=OP.subtract)
                stdt = small.tile([128, gs], F32, name=f"std{g}")
                nc.scalar.activation(stdt[:], var[:], AF.Sqrt, bias=epst[:])
                rstd = small.tile([128, gs], F32, name=f"rstd{g}")
                nc.vector.reciprocal(out=rstd[:], in_=stdt[:])
                acols = ab[:, o0:o0 + gs]
                bcols = ab[:, 16 + o0:16 + o0 + gs]
                nc.vector.tensor_tensor(acols, gmt[:, o0:o0 + gs], rstd[:],
                                        op=OP.mult)
                nc.vector.tensor_tensor(bcols, mean[:], acols, op=OP.mult)
                nc.vector.tensor_tensor(bcols, btt[:, o0:o0 + gs], bcols,
                                        op=OP.subtract)
                for mo in range(gs):
                    m = o0 + mo
                    nc.vector.tensor_scalar(
                        out=y16[:, m, :], in0=y16[:, m, :],
                        scalar1=ab[:, m:m + 1], scalar2=ab[:, 16 + m:17 + m],
                        op0=OP.mult, op1=OP.add)
                yT_t = yT_d[:].rearrange("p (mt b) -> p mt b", b=BS)
                nc.sync.dma_start(yT_t[:, o0:o0 + gs, :],
                                  y16[:, o0:o0 + gs, :])

            # choreography: finalize of group g-1 is emitted inside group g's
            # matmul window so each engine FIFO stays unblocked (see header)
            mm_group(0)
            nc.sync.dma_start(red[:, 0:16], obs[0][:])
            mm_group(1)
            nc.sync.dma_start(red[:, 16:28], obs[1][:])
            fin_group(0)
            mm_group(2)
            fin_group(1)
            nc.sync.dma_start(red[:, 28:32], obs[2][:])
            fin_group(2)

    nc.compile()
    return nc


def _get_nc():
    if "nc" not in _CACHE:
        _CACHE["nc"] = _build_nc()
    return _CACHE["nc"]


def _fold(v):
    """(1, D) or (D,) feature vector -> (128, NIT) column-per-i-tile layout."""
    return np.ascontiguousarray(v.reshape(NIT, 128).T).astype(np.float32)


def kernel(x, scale, translate, wave_weight, base_weight, gamma, beta):
    x = np.asarray(x, dtype=np.float32)
    scale = np.asarray(scale, dtype=np.float32).reshape(1, D)
    translate = np.asarray(translate, dtype=np.float32).reshape(1, D)
    wave_weight = np.asarray(wave_weight, dtype=np.float32)
    base_weight = np.asarray(base_weight, dtype=np.float32)
    gamma = np.asarray(gamma, dtype=np.float32).reshape(D)
    beta = np.asarray(beta, dtype=np.float32).reshape(D)

    inv_s = 1.0 / np.maximum(scale, 1e-3)                     # (1, D)
    # t = x*s3 + b3 = phi/(2pi), phi = 3*(x - tr)*inv_s + pi/2
    s3 = 3.0 * inv_s / TWO_PI
    b3 = (math.pi / 2 - 3.0 * translate * inv_s) / TWO_PI
    # u^2 via Square(x*su + bu), u = (x - tr)*inv_s
    su = inv_s
    bu = -translate * inv_s

    import ml_dtypes
    wcat = np.concatenate([0.3 * base_weight.T,
                           (math.pi ** -0.25) * wave_weight.T], axis=0)
    wcat = (WSCALE * wcat).astype(ml_dtypes.float8_e3m4)       # (2D, D)

    # weights per o-group, chunked 4 k-tiles at a time, per-partition contiguous:
    # w_g[c*128+p, ktl*128*gs + o'] = wcat[(4c+ktl)*128 + p, o0*128 + o']
    w_arrs = []
    for (o0, gs, _) in GROUPS:
        wg = wcat[:, o0 * 128:(o0 + gs) * 128]                 # (4096, 128*gs)
        wg = wg.reshape(NWCH, 4, 128, 128 * gs).transpose(0, 2, 1, 3)
        w_arrs.append(np.ascontiguousarray(
            wg.reshape(NWCH * 128, 4 * 128 * gs)))

    cst = np.concatenate([_fold(s3), _fold(b3), _fold(su), _fold(bu),
                          _fold(gamma), _fold(beta)], axis=1)
    common = dict(w0=w_arrs[0], w1=w_arrs[1], w2=w_arrs[2],
                  cst=np.ascontiguousarray(cst))

    # x per core: xh[p, kt*BS + b] = x[c*BS + b, kt*128 + p]
    in_maps = []
    for c in range(NCORES):
        xc = x[c * BS:(c + 1) * BS, :].astype(np.float16)      # (BS, D)
        xh = xc.T.reshape(NIT, 128, BS).transpose(1, 0, 2)
        in_maps.append(dict(
            xh=np.ascontiguousarray(xh.reshape(128, NIT * BS)), **common))

    nc = _get_nc()
    res = run_bass_kernel_spmd(nc, in_maps, core_ids=list(range(NCORES)),
                               **_CACHE.pop("run_kwargs", {}))
    _CACHE["last_res"] = res
    outs = []
    for c in range(NCORES):
        yt = np.asarray(res.results[c]["yT"]).reshape(128, NOT, BS)
        outs.append(yt.transpose(1, 0, 2).reshape(D, BS))      # (2048, 512)
    yT = np.concatenate(outs, axis=1)                          # (2048, 4096)
    return np.ascontiguousarray(yT.T).astype(np.float32)


# revision 32
# speedup vs baseline: 1.3487x; 1.3487x over previous
"""BioWaveKAN fused kernel for 8 Trainium2 NeuronCores.

y = wavelet(x) @ (pi^-1/4 * Ww).T + x @ (0.3 * Wb).T   (single K=4096 contraction)
out = BatchNorm1d(y)  (training-mode batch stats, all-reduced across cores)

Sharding: data-parallel over batch (8 x 512 rows). Device layout is transposed
(features on partitions); host pre-arranges x/weights into per-partition
contiguous DMA layouts and post-transposes the fp16 output.

v2 structure per core (vs v1: two-pass h0/h1 with a late stats AllReduce):
  - full-K (32 k-tile) PSUM accumulation per o-tile: one drain instead of two
  - o-tiles in 3 groups of [8, 6, 2] (PSUM banks); per-group stats AllReduce +
    finalize + normalize + store interleaved so only the last (2-tile) group's
    collective is exposed in the tail
  - warm-up matmuls keep the PE HAM un-throttled through the DMA fill
  - wavelet: DVE (t, k magic range-reduce, r, sin*exp product) + ACT (Square,
    Exp, then a single-table-switch Sin pass; Sqrt table prefetched);
    GpSimd is avoided for big tiles (~9 G elem/s measured on [128,512])
  - drains fully on DVE (copy+sum / square+sumsq via accum_out), keeping ACT
    off the tail and avoiding activation-table thrash
"""
import math

import numpy as np

from concourse import bacc
import concourse.tile as tile
import concourse.mybir as mybir
from concourse.bass_utils import run_bass_kernel_spmd

F32 = mybir.dt.float32
F16 = mybir.dt.float16
AF = mybir.ActivationFunctionType
OP = mybir.AluOpType

B = 4096          # batch
D = 2048          # in_dim == out_dim
NCORES = 8
BS = B // NCORES  # batch shard per core (512)
NIT = D // 128    # i-tiles (16)
NKT = 2 * NIT     # contraction tiles (32): 0..15 = x, 16..31 = wavelet
NOT = D // 128    # o-tiles (16)
BN_EPS = 1e-5
TWO_PI = 2.0 * math.pi
MAGIC = 1.5 * 2.0 ** 23

# o-tile groups: (o_start, group_size, stats col base); last group small so the
# tail (drain -> AllReduce -> finalize -> store) after the final matmul is short
GROUPS = [(0, 8, 0), (8, 6, 16), (14, 2, 28)]
NWCH = 8          # weight chunks per group (4 k-tiles each)

_CACHE = {}


def _build_nc():
    nc = bacc.Bacc()

    xh_d = nc.dram_tensor("xh", (128, NIT * BS), F16, kind="ExternalInput")
    w_d = [
        nc.dram_tensor(f"w{g}", (NWCH * 128, 4 * 128 * gs), F16,
                       kind="ExternalInput")
        for g, (_, gs, _) in enumerate(GROUPS)
    ]
    cst_d = nc.dram_tensor("cst", (128, 6 * NIT), F32, kind="ExternalInput")
    yT_d = nc.dram_tensor("yT", (128, NOT * BS), F16, kind="ExternalOutput")

    with tile.TileContext(nc) as tc:
        with (
            tc.tile_pool(name="big", bufs=1) as big,
            tc.tile_pool(name="small", bufs=1) as small,
            tc.tile_pool(name="wp0", bufs=6) as wp0,
            tc.tile_pool(name="wp1", bufs=8) as wp1,
            tc.tile_pool(name="wp2", bufs=8) as wp2,
            tc.tile_pool(name="scr", bufs=4) as scr,
            tc.tile_pool(name="qtp", bufs=3) as qtp,
            tc.tile_pool(name="sinp", bufs=2) as sinp,
            tc.tile_pool(name="dscp", bufs=2) as dscp,
            tc.tile_pool(name="ps", bufs=8, space="PSUM") as ps,
            tc.tile_pool(name="dram", bufs=1, space="DRAM") as dram,
        ):
            rhs = big.tile([128, NKT, BS], F16)   # 0..15 x, 16..31 wavelet
            y16 = big.tile([128, NOT, BS], F16)
            et = big.tile([128, NIT, BS], F16)    # exp(-u^2/2) per i-tile
            cstt = small.tile([128, 6 * NIT], F32)
            stats = small.tile([128, 32], F32)    # per group: sums then sumsqs
            red = small.tile([128, 32], F32)
            ab = small.tile([128, 32], F32)       # a cols 0..15, b cols 16..31
            wz = small.tile([128, 64], F16)

            zbt = small.tile([128, 1], F32)
            nc.vector.memset(zbt[:], 0.0)
            epst = small.tile([128, 1], F32)
            nc.vector.memset(epst[:], BN_EPS)

            # ---- PE warm-up: keep HAM at K=8/8 through the DMA fill ----
            nc.vector.memset(wz[:], 0.0)
            psw = ps.tile([128, BS], F32, tag="ps", name="psw")
            for _ in range(80):
                nc.tensor.matmul(psw[0:1, 0:64], wz[:, 0:1], wz[:, 0:64],
                                 start=True, stop=True)

            # ---- DMA issue order: consts, x, weights in consumption order ----
            nc.sync.dma_start(cstt[:], cst_d[:])
            s3t = cstt[:, 0 * NIT:1 * NIT]
            b3t = cstt[:, 1 * NIT:2 * NIT]
            sut = cstt[:, 2 * NIT:3 * NIT]
            but = cstt[:, 3 * NIT:4 * NIT]
            gmt = cstt[:, 4 * NIT:5 * NIT]
            btt = cstt[:, 5 * NIT:6 * NIT]

            xh_t = xh_d[:].rearrange("p (kt b) -> p kt b", b=BS)
            nc.sync.dma_start(rhs[:, 0:4, :], xh_t[:, 0:4, :])

            wtiles = {g: [] for g in range(len(GROUPS))}

            def issue_w(g, c, eng):
                gs = GROUPS[g][1]
                wt = locals_pools[g].tile([128, 4, 128 * gs], F16, tag=f"w{g}",
                                          name=f"w{g}_{c}")
                eng.dma_start(
                    wt[:], w_d[g][c * 128:(c + 1) * 128, :].rearrange(
                        "p (k o) -> p k o", k=4))
                wtiles[g].append(wt)

            locals_pools = {0: wp0, 1: wp1, 2: wp2}
            issue_w(0, 0, nc.sync)
            for c in range(1, 4):
                nc.sync.dma_start(rhs[:, c * 4:(c + 1) * 4, :],
                                  xh_t[:, c * 4:(c + 1) * 4, :])
            for c in range(1, NWCH):
                issue_w(0, c, nc.sync)
            for c in range(NWCH):
                issue_w(1, c, nc.sync)
            for c in range(NWCH):
                issue_w(2, c, nc.sync)

            # bridge warm-ups: depend on the x DMA so they run while the
            # first weight chunk is still in flight, keeping HAM at 8/8
            # (a >3.4us PE idle gap re-throttles the PE clock to 4/8)
            for j in range(40):
                nc.tensor.matmul(psw[0:1, 0:64], wz[:, 0:1],
                                 rhs[:, j % 4, 0:64], start=True, stop=True)

            # ---- wavelet: rhs[:,16+i,:] = sin(2*pi*frac(t)) * exp(-u^2/2) ----
            # t,k (magic-number range reduction), r on DVE;
            # Square/Exp then Sin on ACT (one table switch)
            for i in range(NIT):
                xf = rhs[:, i, :]
                tt_ = scr.tile([128, BS], F32, tag="scr", name=f"t_{i}")
                nc.vector.tensor_scalar(out=tt_[:], in0=xf,
                                        scalar1=s3t[:, i:i + 1],
                                        scalar2=b3t[:, i:i + 1],
                                        op0=OP.mult, op1=OP.add)
                kt_ = scr.tile([128, BS], F32, tag="scr", name=f"k_{i}")
                nc.vector.tensor_scalar(out=kt_[:], in0=tt_[:],
                                        scalar1=MAGIC, scalar2=MAGIC,
                                        op0=OP.add, op1=OP.subtract)
                nc.vector.tensor_tensor(rhs[:, NIT + i, :], tt_[:], kt_[:],
                                        op=OP.subtract)
                qt = qtp.tile([128, BS], F32, tag="qt", name=f"q_{i}")
                nc.scalar.activation(qt[:], xf, AF.Square,
                                     bias=but[:, i:i + 1], scale=sut[:, i:i + 1])
                nc.scalar.activation(et[:, i, :], qt[:], AF.Exp,
                                     bias=zbt[:], scale=-0.5)
            for i in range(NIT):
                sint = sinp.tile([128, BS], F16, tag="sin", name=f"s_{i}")
                nc.scalar.activation(sint[:], rhs[:, NIT + i, :], AF.Sin,
                                     bias=zbt[:], scale=TWO_PI)
                nc.vector.tensor_tensor(rhs[:, NIT + i, :], sint[:],
                                        et[:, i, :], op=OP.mult)
            # prefetch the sqrt table set off the critical path
            sqp = small.tile([128, 1], F32)
            nc.scalar.activation(sqp[:], zbt[:], AF.Sqrt, bias=epst[:])

            ibs, obs = {}, {}
            for g, (_, gs, _) in enumerate(GROUPS):
                ibs[g] = dram.tile([128, 2 * gs], F32, name=f"ib{g}")
                obs[g] = dram.tile([128, 2 * gs], F32, name=f"ob{g}")

            def mm_group(g):
                o0, gs, cb = GROUPS[g]
                psums = []
                for mo in range(gs):
                    pst = ps.tile([128, BS], F32, tag="ps",
                                  name=f"ps_{g}_{mo}")
                    psums.append(pst)
                for kt in range(NKT):
                    ch = wtiles[g][kt // 4]
                    ktl = kt % 4
                    for mo in range(gs):
                        nc.tensor.matmul(
                            psums[mo][:],
                            ch[:, ktl, mo * 128:(mo + 1) * 128],
                            rhs[:, kt, :],
                            start=(kt == 0), stop=(kt == NKT - 1))
                # fused drains on DVE: y16 = psum (fp16) + sum / sumsq accums
                for mo in range(gs):
                    m = o0 + mo
                    nc.vector.tensor_scalar(
                        out=y16[:, m, :], in0=psums[mo][:],
                        scalar1=1.0, scalar2=0.0, op0=OP.mult, op1=OP.add,
                        accum_out=stats[:, cb + mo:cb + mo + 1])
                    dsc = dscp.tile([128, BS], F16, tag="dsc", name=f"d_{m}")
                    nc.vector.scalar_tensor_tensor(
                        out=dsc[:], in0=y16[:, m, :], scalar=1.0,
                        in1=y16[:, m, :], op0=OP.mult, op1=OP.mult,
                        accum_out=stats[:, cb + gs + mo:cb + gs + mo + 1])
                nc.sync.dma_start(ibs[g][:], stats[:, cb:cb + 2 * gs])
                nc.gpsimd.collective_compute(
                    "AllReduce", OP.add,
                    replica_groups=[list(range(NCORES))],
                    ins=[ibs[g].opt()], outs=[obs[g].opt()])

            def fin_group(g):
                o0, gs, cb = GROUPS[g]
                rs = red[:, cb:cb + gs]          # sums
                rq = red[:, cb + gs:cb + 2 * gs]  # sumsqs
                mean = small.tile([128, gs], F32, name=f"mean{g}")
                nc.vector.tensor_scalar(out=mean[:], in0=rs, scalar1=1.0 / B,
                                        scalar2=None, op0=OP.mult)
                var = small.tile([128, gs], F32, name=f"var{g}")
                nc.vector.tensor_scalar(out=var[:], in0=rq, scalar1=1.0 / B,
                                        scalar2=None, op0=OP.mult)
                msq = small.tile([128, gs], F32, name=f"msq{g}")
                nc.vector.tensor_tensor(msq[:], mean[:], mean[:], op=OP.mult)
                nc.vector.tensor_tensor(var[:], var[:], msq[:], op=OP.subtract)
                stdt = small.tile([128, gs], F32, name=f"std{g}")
                nc.scalar.activation(stdt[:], var[:], AF.Sqrt, bias=epst[:])
                rstd = small.tile([128, gs], F32, name=f"rstd{g}")
                nc.vector.reciprocal(out=rstd[:], in_=stdt[:])
                acols = ab[:, o0:o0 + gs]
                bcols = ab[:, 16 + o0:16 + o0 + gs]
                nc.vector.tensor_tensor(acols, gmt[:, o0:o0 + gs], rstd[:],
                                        op=OP.mult)
                nc.vector.tensor_tensor(bcols, mean[:], acols, op=OP.mult)
                nc.vector.tensor_tensor(bcols, btt[:, o0:o0 + gs], bcols,
                                        op=OP.subtract)
                for mo in range(gs):
                    m = o0 + mo
                    nc.vector.tensor_scalar(
                        out=y16[:, m, :], in0=y16[:, m, :],
                        scalar1=ab[:, m:m + 1], scalar2=ab[:, 16 + m:17 + m],
                        op0=OP.mult, op1=OP.add)
                yT_t = yT_d[:].rearrange("p (mt b) -> p mt b", b=BS)
                nc.sync.dma_start(yT_t[:, o0:o0 + gs, :],
                                  y16[:, o0:o0 + gs, :])

            # choreography: finalize of group g-1 is emitted inside group g's
            # matmul window so each engine FIFO stays unblocked (see header)
            mm_group(0)
            nc.sync.dma_start(red[:, 0:16], obs[0][:])
            mm_group(1)
            nc.sync.dma_start(red[:, 16:28], obs[1][:])
            fin_group(0)
            mm_group(2)
            fin_group(1)
            nc.sync.dma_start(red[:, 28:32], obs[2][:])
            fin_group(2)

    nc.compile()
    return nc


def _get_nc():
    if "nc" not in _CACHE:
        _CACHE["nc"] = _build_nc()
    return _CACHE["nc"]


def _fold(v):
    """(1, D) or (D,) feature vector -> (128, NIT) column-per-i-tile layout."""
    return np.ascontiguousarray(v.reshape(NIT, 128).T).astype(np.float32)


def kernel(x, scale, translate, wave_weight, base_weight, gamma, beta):
    x = np.asarray(x, dtype=np.float32)
    scale = np.asarray(scale, dtype=np.float32).reshape(1, D)
    translate = np.asarray(translate, dtype=np.float32).reshape(1, D)
    wave_weight = np.asarray(wave_weight, dtype=np.float32)
    base_weight = np.asarray(base_weight, dtype=np.float32)
    gamma = np.asarray(gamma, dtype=np.float32).reshape(D)
    beta = np.asarray(beta, dtype=np.float32).reshape(D)

    inv_s = 1.0 / np.maximum(scale, 1e-3)                     # (1, D)
    # t = x*s3 + b3 = phi/(2pi), phi = 3*(x - tr)*inv_s + pi/2
    s3 = 3.0 * inv_s / TWO_PI
    b3 = (math.pi / 2 - 3.0 * translate * inv_s) / TWO_PI
    # u^2 via Square(x*su + bu), u = (x - tr)*inv_s
    su = inv_s
    bu = -translate * inv_s

    wcat = np.concatenate([0.3 * base_weight.T,
                           (math.pi ** -0.25) * wave_weight.T], axis=0)
    wcat = wcat.astype(np.float16)                             # (2D, D)

    # weights per o-group, chunked 4 k-tiles at a time, per-partition contiguous:
    # w_g[c*128+p, ktl*128*gs + o'] = wcat[(4c+ktl)*128 + p, o0*128 + o']
    w_arrs = []
    for (o0, gs, _) in GROUPS:
        wg = wcat[:, o0 * 128:(o0 + gs) * 128]                 # (4096, 128*gs)
        wg = wg.reshape(NWCH, 4, 128, 128 * gs).transpose(0, 2, 1, 3)
        w_arrs.append(np.ascontiguousarray(
            wg.reshape(NWCH * 128, 4 * 128 * gs)))

    cst = np.concatenate([_fold(s3), _fold(b3), _fold(su), _fold(bu),
                          _fold(gamma), _fold(beta)], axis=1)
    common = dict(w0=w_arrs[0], w1=w_arrs[1], w2=w_arrs[2],
                  cst=np.ascontiguousarray(cst))

    # x per core: xh[p, kt*BS + b] = x[c*BS + b, kt*128 + p]
    in_maps = []
    for c in range(NCORES):
        xc = x[c * BS:(c + 1) * BS, :].astype(np.float16)      # (BS, D)
        xh = xc.T.reshape(NIT, 128, BS).transpose(1, 0, 2)
        in_maps.append(dict(
            xh=np.ascontiguousarray(xh.reshape(128, NIT * BS)), **common))

    nc = _get_nc()
    res = run_bass_kernel_spmd(nc, in_maps, core_ids=list(range(NCORES)),
                               **_CACHE.pop("run_kwargs", {}))
    _CACHE["last_res"] = res
    outs = []
    for c in range(NCORES):
        yt = np.asarray(res.results[c]["yT"]).reshape(128, NOT, BS)
        outs.append(yt.transpose(1, 0, 2).reshape(D, BS))      # (2048, 512)
    yT = np.concatenate(outs, axis=1)                          # (2048, 4096)
    return np.ascontiguousarray(yT.T).astype(np.float32)
